# revision 4
# baseline (speedup 1.0000x reference)
"""Trainium2 Bass kernel for nn_MessageFunction (GNN message passing).

Design (hardware-measured on trn2, sustained ~1.93GHz PE clock):
- fp16 matmuls, fp32 PSUM; 8 blocks x 6 m-tiles x 12-matmul groups,
  8 PSUM banks; inputs/weights host-packed partition-major so every
  DMA walks HBM linearly; output stored fp16 in instruction order and
  host-unpacked to [BPC, D, N] f32 (halves output traffic).
- Weights + bias loaded once before the loop and kept SBUF-resident;
  block 0 loads per-k each pass so the first matmul group waits on
  2x128KB, not 2x768KB. Rings: e/sync, h/scalar, out/gpsimd.
"""
import numpy as np
import concourse.tile as tile
from concourse import bacc, mybir
from concourse.bass_utils import run_bass_kernel_spmd

try:
    import jax
    jax.config.update("jax_compilation_cache_dir", "/tmp/.jax_kernel_cache")
    jax.config.update("jax_persistent_cache_min_compile_time_secs", 0.5)
except Exception:
    pass

B, D, NN = 128, 768, 256
NCORES = 8
BPC = B // NCORES
PAIR = 2
NBLK = BPC // PAIR
NCOL = PAIR * NN
KT = D // 128
MT = D // 128
F32 = mybir.dt.float32
F16 = mybir.dt.float16
DT = F16
NPDT = np.float16


def build(repeat: int = 1, loop_repeat: int = 1):
    nc = bacc.Bacc("TRN2", target_bir_lowering=False, debug=False,
                   num_devices=NCORES)
    e = nc.dram_tensor("e", [NBLK, 128, KT, NCOL], DT, kind="ExternalInput").ap()
    h = nc.dram_tensor("h", [NBLK, 128, KT, NCOL], DT, kind="ExternalInput").ap()
    wep = nc.dram_tensor("wep", [MT, 128, KT, 128], DT, kind="ExternalInput").ap()
    wwp = nc.dram_tensor("wwp", [MT, 128, KT, 128], DT, kind="ExternalInput").ap()
    biasp = nc.dram_tensor("biasp", [128, MT], F32, kind="ExternalInput").ap()
    out = nc.dram_tensor("out", [NBLK, MT, 128, NCOL], F16,
                         kind="ExternalOutput").ap()

    with tile.TileContext(nc) as tc:
        with (
            tc.tile_pool(name="wpool", bufs=1) as wpool,
            tc.tile_pool(name="xpool", bufs=3) as xpool,
            tc.tile_pool(name="opool", bufs=6) as opool,
            tc.tile_pool(name="pspool", bufs=8, space="PSUM") as pspool,
        ):
            we_t = wpool.tile([128, MT, KT, 128], DT)
            ww_t = wpool.tile([128, MT, KT, 128], DT)
            bias_t = wpool.tile([128, MT], F32)
            # weights + bias loaded once, before the loop; resident after.
            nc.sync.dma_start(bias_t[:], biasp)
            for m in range(MT):
                nc.sync.dma_start(we_t[:, m], wep[m])
                nc.scalar.dma_start(ww_t[:, m], wwp[m])

            def _block(c):
                et = xpool.tile([128, KT, NCOL], DT, tag="et", name="et")
                ht = xpool.tile([128, KT, NCOL], DT, tag="ht", name="ht")
                if c == 0:
                    # per-k loads: the pass's first matmul group starts
                    # after 2x128KB of DMA (subtile deps gate each k).
                    for k in range(KT):
                        nc.sync.dma_start(et[:, k], e[c, :, k])
                        nc.scalar.dma_start(ht[:, k], h[c, :, k])
                else:
                    nc.sync.dma_start(et[:], e[c])
                    nc.scalar.dma_start(ht[:], h[c])
                for m in range(MT):
                    ps = pspool.tile([128, NCOL], F32, name="ps")
                    for k in range(KT):
                        nc.tensor.matmul(ps[:], we_t[:, m, k], et[:, k],
                                         start=(k == 0), stop=False)
                    for k in range(KT):
                        nc.tensor.matmul(ps[:], ww_t[:, m, k], ht[:, k],
                                         start=False, stop=(k == KT - 1))
                    res = opool.tile([128, NCOL], F16, name="res")
                    nc.scalar.activation(
                        res[:], ps[:], mybir.ActivationFunctionType.Identity,
                        bias=bias_t[:, m:m + 1], scale=1.0)
                    nc.gpsimd.dma_start(out[c, m], res[:])

            def body():
                for _ in range(repeat):
                    for c in range(NBLK):
                        _block(c)

            if loop_repeat > 1:
                with tc.For_i(0, loop_repeat, 1,
                              hint_engines=(mybir.EngineType.PE,)):
                    body()
            else:
                body()
    nc.compile()
    return nc


def _prep_in_maps(h_w, e_vw, We, be, Ww, bw):
    e_vw = np.asarray(e_vw, dtype=np.float32).astype(NPDT)
    h_w = np.asarray(h_w, dtype=np.float32).astype(NPDT)

    def wpack(W):
        # W[o,d] -> wp[m,p,k,q] = W[m*128+q, k*128+p]  (lhsT = W.T tiles)
        wT = np.asarray(W, dtype=np.float32).T.astype(NPDT)  # [d, o]
        return np.ascontiguousarray(
            wT.reshape(KT, 128, MT, 128).transpose(2, 1, 0, 3))

    bias = (np.asarray(be, dtype=np.float32)
            + np.asarray(bw, dtype=np.float32)).astype(np.float32)
    biasp = np.ascontiguousarray(bias.reshape(MT, 128).T)   # [128, MT]

    def slab(x, c):
        # [BPC,D,NN] -> [NBLK,128,KT,NCOL]: s[c,p,k,j*NN+n] = x[c*PAIR+j, k*128+p, n]
        s = x[c * BPC:(c + 1) * BPC].reshape(NBLK, PAIR, KT, 128, NN)
        return np.ascontiguousarray(
            s.transpose(0, 3, 2, 1, 4).reshape(NBLK, 128, KT, NCOL))

    wep, wwp = wpack(We), wpack(Ww)
    return [
        {"e": slab(e_vw, c), "h": slab(h_w, c),
         "wep": wep, "wwp": wwp, "biasp": biasp}
        for c in range(NCORES)
    ]


def _unpack_out(res):
    # [NBLK,MT,128,NCOL] f16 -> [BPC,D,NN] f32
    s = res.reshape(NBLK, MT, 128, PAIR, NN).astype(np.float32)
    return s.transpose(0, 3, 1, 2, 4).reshape(BPC, D, NN)


_NC_CACHE = []


def kernel(h_v, h_w, e_vw, We, be, Ww, bw):
    if not _NC_CACHE:
        _NC_CACHE.append(build())
    nc = _NC_CACHE[0]
    in_maps = _prep_in_maps(h_w, e_vw, We, be, Ww, bw)
    r = run_bass_kernel_spmd(nc, in_maps, core_ids=list(range(NCORES)))
    return np.concatenate(
        [_unpack_out(r.results[c]["out"]) for c in range(NCORES)], axis=0)
